# revision 38
# baseline (speedup 1.0000x reference)
"""Multi-head attention TRN2 kernel.

Full inputs -> 8-core shard (batch x head-group) -> Bass/Tile kernel -> host
gather+reduce.

Problem (hardcoded): B=2, S=2048, D_MODEL=1024, H=16, Dk=64, fp32, mask=0.

Sharding: core c = b*4 + g handles batch b and head group g (4 heads).
Each core computes, for its batch's full sequence:
  QT/KT = (x @ Wq_g)^T, V = x @ Wv_g            (x transposed on host, bf16)
  scoresT[j,i] = K Q^T  per head pair            (row-tiled on the PE array)
  attnT = exp(scoresT/8)                         (scalar engine, psum->bf16)
  ctx[i,dk]  = sum_j attnT V: attn chunks are the STATIONARY operand and the
               (dk+ones) V columns are the 65-row moving operand, so the PE
               pays 65 rows per 128x128 attn chunk instead of 512.  The ones
               column makes the softmax denominator ride along in psum.
  normalize: DVE tensor_scalar multiply by the per-partition reciprocal of
             the denominator column (fused with the psum->sbuf move).
  ctx^T via DMA-engine transposes (bf16 xbar path, zero PE cost).
  partial_out = ctxT^T @ Wo_g -> bf16 partials, host sums the 4 group
               partials per batch in f32.
All matmul moving operands are bf16 (1 PE cycle/row in the cost model at any
size).  psum->sbuf moves for projections and the output ride on the
otherwise-idle Pool (gpsimd) engine so the DVE only runs the normalize.
"""

import sys

import ml_dtypes
import numpy as np

try:
    import concourse.bass as bass  # noqa: F401
except ImportError:  # harness runs from a bare directory
    sys.path.insert(0, "/opt/trn_rl_repo")
    import concourse.bass as bass  # noqa: F401

import concourse.tile as tile
from concourse import bacc, mybir
from concourse.bass_utils import run_bass_kernel_spmd
from concourse.masks import make_identity

S = 2048
D = 1024
HG = 4  # heads per core
DK = 64
DKV = HG * DK  # 256
P = 128
F32 = mybir.dt.float32
BF16 = mybir.dt.bfloat16
EXP = mybir.ActivationFunctionType.Exp

_NC_CACHE = []


def _build_nc():
    nc = bacc.Bacc("TRN2", target_bir_lowering=False, debug=False)
    xqT = nc.dram_tensor("xqT", [D, S], BF16, kind="ExternalInput")
    xkT = nc.dram_tensor("xkT", [D, S], BF16, kind="ExternalInput")
    xvT = nc.dram_tensor("xvT", [D, S], BF16, kind="ExternalInput")
    wq = nc.dram_tensor("wq", [D, DKV], BF16, kind="ExternalInput")
    wk = nc.dram_tensor("wk", [D, DKV], BF16, kind="ExternalInput")
    wv = nc.dram_tensor("wv", [D, DKV], BF16, kind="ExternalInput")
    wo = nc.dram_tensor("wo", [DKV, D], BF16, kind="ExternalInput")
    out = nc.dram_tensor("out", [S, D], BF16, kind="ExternalOutput")

    with tile.TileContext(nc) as tc:
        with (
            tc.tile_pool(name="persist", bufs=1) as persist,
            tc.tile_pool(name="xin", bufs=38) as xin,
            tc.tile_pool(name="attn", bufs=31) as attn_pool,
            tc.tile_pool(name="small", bufs=8) as small_pool,
            tc.tile_pool(name="hid", bufs=6) as hid_pool,
            tc.tile_pool(name="sc_ps", bufs=2, space="PSUM") as sc_ps,
            tc.tile_pool(name="ctx_ps", bufs=2, space="PSUM") as ctx_ps,
            tc.tile_pool(name="wo_ps", bufs=2, space="PSUM") as wo_ps,
        ):
            # Persistent SBUF tensors.
            # QT/KT: [dkv%128, pair, i] -- partition r = (h%2)*64 + dk.
            QT = persist.tile([P, 2, S], BF16)
            KT = persist.tile([P, 2, S], BF16)
            # V augmented with a ones column (-> softmax denominator rides
            # along in the ctx matmul): [j%128, jc, head, dk+1].
            Vag = persist.tile([P, 16, HG, DK + 1], BF16)
            # Normalized ctx in query-major layout: [i%128, ichunk, pair,
            # head01*64+dk]; transposed (DMA xbar) into ctxT [dkv%128, pair, i]
            ctxN = persist.tile([P, 16, 2, P], BF16)
            ctxT = persist.tile([P, 2, S], BF16)
            wq_sb = persist.tile([P, 8, DKV], BF16)
            wk_sb = persist.tile([P, 8, DKV], BF16)
            wv_sb = persist.tile([P, 8, DKV], BF16)
            wo_sb = persist.tile([P, 2, D], BF16)

            # Preload the exp table set (~1.3us) during the DMA head.
            warm = small_pool.tile([P, 8], F32, tag="warm")
            nc.vector.memset(warm[0:1, :], 0.0)
            nc.scalar.activation(warm[0:1, :], warm[0:1, :], EXP, scale=0.0)
            # Ones column of Vag.
            nc.vector.memset(Vag[:, :, :, DK], 1.0)
            # Identity for the last block's PE-path transposes.
            ident = persist.tile([P, P], BF16)
            make_identity(nc, ident)

            def alloc_block(name):
                """Reserve the 8 k-chunk tiles for one 1024-wide block.
                The xv blocks are allocated FIRST (before the longer-lived
                xk/xq blocks) so the pool's FIFO reuse hands their slots --
                dead after block (0,0)'s V passes -- to the tail of the xk1/
                xq1 streams instead of stalling on pair-1-pinned chunks."""
                return [
                    xin.tile([P, 1024], BF16, tag="xt", name=f"{name}{k}")
                    for k in range(8)
                ]

            def load_block(xT, icb, xts):
                """Issue the 8 k-chunk DMAs for one 1024-wide column block."""
                for k in range(8):
                    nc.sync.dma_start(
                        xts[k],
                        xT[k * P : (k + 1) * P, icb * 1024 : (icb + 1) * 1024],
                    )
                return xts

            # [pair][512-block] flags marking which Q/K projection passes
            # have been EMITTED -- the exp pump must not run ahead of them
            # (an in-order PE stream would deadlock on the missing pass).
            qt_done = [[False] * 4 for _ in range(2)]
            kt_done = [[False] * 4 for _ in range(2)]

            def proj_qk_pass(xts, w_sb, OUT, pair, ib, ks=range(8), st=None):
                """Project one (pair, 512-wide i block) of x into OUT."""
                icb, ic2 = ib // 2, ib % 2
                if st is None or "acc" not in st:
                    acc = wo_ps.tile([P, 512], F32, tag="wo", name="pacc")
                    if st is not None:
                        st["acc"] = acc
                else:
                    acc = st.pop("acc")
                for k in ks:
                    nc.tensor.matmul(
                        acc,
                        w_sb[:, k, pair * P : (pair + 1) * P],
                        xts[k][:, ic2 * 512 : (ic2 + 1) * 512],
                        start=(k == 0),
                        stop=(k == 7),
                    )
                if ks[-1] == 7:
                    o0 = ib * 512
                    nc.vector.tensor_copy(OUT[:, pair, o0 : o0 + 512], acc)
                    done = qt_done if OUT is QT else kt_done
                    done[pair][ib] = True

            def qk_halves(extras, jc, xts, w_sb, OUT, pair, ib):
                """Weave one projection pass as two 4-matmul halves in
                adjacent jc slots (the half-open psum accumulator tolerates
                at most one other pool allocation in between, which the
                placements below respect)."""
                st = {}
                extras.setdefault(jc, []).append(
                    lambda: proj_qk_pass(xts, w_sb, OUT, pair, ib, range(4), st)
                )
                extras.setdefault(jc + 1, []).append(
                    lambda: proj_qk_pass(xts, w_sb, OUT, pair, ib, range(4, 8), st)
                )
                return extras

            def proj_v_pass(xts, jc):
                """Project one 128-wide j chunk of value into Vag[:, jc]."""
                c0 = (jc % 8) * P
                acc = wo_ps.tile([P, 512], F32, tag="wo", name="vacc")
                for k in range(8):
                    nc.tensor.matmul(
                        acc[:, 0:DKV],
                        xts[k][:, c0 : c0 + P],
                        wv_sb[:, k, :],
                        start=(k == 0),
                        stop=(k == 7),
                    )
                nc.vector.tensor_copy(
                    Vag[:, jc, :, 0:DK],
                    acc[:, 0:DKV].rearrange("p (h d) -> p h d", h=HG),
                )

            def emit_scores(ic, pair, jc):
                """scoresT for the head pair (row-tiled, concurrent) + exp."""
                i0 = ic * 512
                j0 = jc * P
                sc = sc_ps.tile([P, 1024], F32, tag="sc", name="sc")
                nc.tensor.matmul(
                    sc[:, 0:512],
                    KT[0:64, pair, j0 : j0 + P],
                    QT[0:64, pair, i0 : i0 + 512],
                    start=True,
                    stop=True,
                )
                nc.tensor.matmul(
                    sc[:, 512:1024],
                    KT[64:128, pair, j0 : j0 + P],
                    QT[64:128, pair, i0 : i0 + 512],
                    start=True,
                    stop=True,
                )
                at = attn_pool.tile([P, 1024], BF16, tag="at")
                nc.scalar.activation(at, sc, EXP, scale=0.125)
                return at

            def emit_ctx(ctx_e, ctx_o, pair, jc, at):
                """ctx accumulation: attn chunks stationary, (V|1) moving.

                psum region sub*65..sub*65+65 of ctx_e/ctx_o holds the i
                sub-chunk's (dk + denominator) columns.  Only the first
                region's jc==0 matmul uses start=True: it zeroes the whole
                2KB psum bank, which covers the other three regions (trn2
                zero-region == bank)."""
                for head01, bank in ((0, ctx_e), (1, ctx_o)):
                    h = 2 * pair + head01
                    for sub in range(4):
                        first = jc == 0 and sub == 0
                        nc.tensor.matmul(
                            bank[:, sub * 65 : sub * 65 + 65],
                            at[:, head01 * 512 + sub * P : head01 * 512 + (sub + 1) * P],
                            Vag[:, jc, h, :],
                            start=first,
                            stop=(jc == 15 and sub == 3),
                            skip_group_check=not first,
                        )

            # Decoupled exp stream: scores+exp for unit u (block-major order)
            # are emitted ("pumped") up to MAXLEAD units ahead of the ctx
            # cursor, so the scalar engine streams exps continuously through
            # the PE-bound projection phase instead of running lockstep with
            # ctx.  The at ring (bufs > MAXLEAD) buffers the lead; psum sc
            # ring pacing self-regulates PE emission against ACT execution.
            # Pair-major block order: all pair-0 blocks run first, so the
            # pair-1 K/Q projection passes move out of the PE-bound front
            # into the later ACT-bound blocks where the PE has slack.
            BLOCKS = [(ic, 0) for ic in range(4)] + [(ic, 1) for ic in range(4)]
            UNITS = [
                (ic, pair, jc) for ic, pair in BLOCKS for jc in range(16)
            ]
            MAXLEAD = 28
            exp_cur = [0]
            pending_at = {}

            def unit_ready(u):
                ic, p, jc = UNITS[u]
                return qt_done[p][ic] and kt_done[p][jc // 4]

            def pump():
                u = exp_cur[0]
                if u >= len(UNITS) or not unit_ready(u):
                    return False
                pending_at[UNITS[u]] = emit_scores(*UNITS[u])
                exp_cur[0] += 1
                return True

            def attn_jcs(ic, pair, ctx_e, ctx_o, extras=None, base_unit=0):
                extras = extras or {}
                for jc in range(16):
                    u = base_unit + jc
                    while exp_cur[0] <= u:
                        assert pump(), f"pump stuck at unit {exp_cur[0]}"
                    # 3 pumps/slot through the PE-bound projection front so
                    # the scalar engine stays fed; 2 in steady state.
                    budget = 3 if u < 32 else 2
                    while budget and exp_cur[0] < min(u + MAXLEAD, len(UNITS)):
                        if not pump():
                            break
                        budget -= 1
                    at = pending_at.pop((ic, pair, jc))
                    emit_ctx(ctx_e, ctx_o, pair, jc, at)
                    for fn in extras.get(jc, ()):
                        fn()

            def attn_norm(ic, pair, ctx_e, ctx_o, pe_tr=False):
                """softmax normalize: per-partition reciprocal of the psum
                denominator columns (one strided op per bank), broadcast-
                multiplied while moving the ctx psum regions to ctxN (bf16);
                then transpose into ctxT.  Transposes ride the DMA xbar path
                (zero PE cost) except for the last block, where the shorter
                PE-transpose latency chain trims the kernel tail."""
                for head01, bank in ((0, ctx_e), (1, ctx_o)):
                    regs = bank[:, 0 : 4 * 65].rearrange("p (s c) -> p s c", c=65)
                    rc4 = small_pool.tile([P, 4], F32, tag="rc")
                    nc.vector.reciprocal(rc4, regs[:, :, DK])
                    with nc.allow_low_precision("bf16 ctx storage"):
                        nc.vector.tensor_mul(
                            ctxN[
                                :,
                                ic * 4 : ic * 4 + 4,
                                pair,
                                head01 * DK : (head01 + 1) * DK,
                            ],
                            regs[:, :, 0:DK],
                            rc4.unsqueeze(2).broadcast_to([P, 4, DK]),
                        )
                if pe_tr:
                    tr = ctx_ps.tile([P, 512], F32, tag="ctx", name="tr")
                    tr_bf = tr.bitcast(BF16)
                    for sub in range(4):
                        nc.tensor.transpose(
                            tr_bf[:, sub * P : (sub + 1) * P],
                            ctxN[:, ic * 4 + sub, pair, :],
                            ident,
                        )
                    nc.vector.tensor_copy(
                        ctxT[:, pair, ic * 512 : (ic + 1) * 512], tr_bf[:, 0:512]
                    )
                    return
                for sub in range(4):
                    ichunk = ic * 4 + sub
                    nc.sync.dma_start_transpose(
                        ctxT[:, pair, ichunk * P : (ichunk + 1) * P],
                        ctxN[:, ichunk, pair, :],
                    )

            def attn_block(ic, pair, extras=None, pe_tr=False):
                with nc.named_scope(f"attn_i{ic}_p{pair}"):
                    ctx_e = ctx_ps.tile([P, 512], F32, tag="ctx", name="ctx_e")
                    ctx_o = ctx_ps.tile([P, 512], F32, tag="ctx", name="ctx_o")
                    attn_jcs(
                        ic, pair, ctx_e, ctx_o, extras,
                        base_unit=BLOCKS.index((ic, pair)) * 16,
                    )
                    attn_norm(ic, pair, ctx_e, ctx_o, pe_tr=pe_tr)

            hs_tiles = {}

            def wo_piece(ichunk, d2, alt=False):
                """One output-projection piece.  The two d halves of an i
                chunk share one [P, 1024] staging tile and a single full-row
                output DMA (2KB contiguous rows).  alt=True (tail only)
                draws the psum accumulator from the retired scores pool and
                runs the psum->sbuf move on the (by then idle) scalar
                engine, so the tail drain pipelines across two psum banks
                and two copy engines."""
                r0 = ichunk * P
                if alt:
                    hp = sc_ps.tile([P, 1024], F32, tag="sc", name="hp")[:, 0:512]
                else:
                    hp = wo_ps.tile([P, 512], F32, tag="wo", name="hp")
                nc.tensor.matmul(
                    hp,
                    ctxT[:, 0, r0 : r0 + P],
                    wo_sb[:, 0, d2 * 512 : (d2 + 1) * 512],
                    start=True,
                    stop=False,
                )
                nc.tensor.matmul(
                    hp,
                    ctxT[:, 1, r0 : r0 + P],
                    wo_sb[:, 1, d2 * 512 : (d2 + 1) * 512],
                    start=False,
                    stop=True,
                )
                if d2 == 0:
                    hs_tiles[ichunk] = hid_pool.tile(
                        [P, 1024], BF16, tag="hs", name="hs"
                    )
                hs = hs_tiles.pop(ichunk) if d2 == 1 else hs_tiles[ichunk]
                if alt:
                    nc.scalar.copy(hs[:, d2 * 512 : (d2 + 1) * 512], hp)
                else:
                    nc.vector.tensor_copy(hs[:, d2 * 512 : (d2 + 1) * 512], hp)
                if d2 == 1:
                    nc.sync.dma_start(out[r0 : r0 + P, :], hs)

            def wo_sched(extras, ic, lo, hi, jcs=(3, 7, 11, 15)):
                """Weave pieces lo..hi of i block ic at the given jc slots
                (the jc15 slot doubles as the block-boundary filler that
                covers the normalize latency)."""
                for n, jc in zip(range(lo, hi), jcs):
                    extras.setdefault(jc, []).append(
                        lambda ichunk=ic * 4 + n // 2, d2=n % 2: wo_piece(ichunk, d2)
                    )
                return extras

            # ---------------- emission schedule ----------------
            # ALL input DMAs are issued up front in arrival-priority order.
            # The xin ring (20 slots) paces them against consumption.
            with nc.named_scope("loads"):
                # Pair-0 weight halves lead the stream (they gate the first
                # projections); pair-1 halves are not consumed until the
                # pair-1 blocks in the back half of the kernel.
                xv01 = alloc_block("xv01_")
                xv23 = alloc_block("xv23_")
                xk0 = alloc_block("xk0_")
                xq0 = alloc_block("xq0_")
                xk1 = alloc_block("xk1_")
                xq1 = alloc_block("xq1_")
                wkr = wk.rearrange("(ko p) n -> p ko n", p=P)
                wqr = wq.rearrange("(ko p) n -> p ko n", p=P)
                nc.sync.dma_start(wk_sb[:, :, 0:P], wkr[:, :, 0:P])
                load_block(xkT, 0, xk0)
                nc.sync.dma_start(wq_sb[:, :, 0:P], wqr[:, :, 0:P])
                load_block(xqT, 0, xq0)
                nc.sync.dma_start(wv_sb, wv.rearrange("(ko p) n -> p ko n", p=P))
                load_block(xvT, 0, xv01)
                load_block(xkT, 1, xk1)
                load_block(xvT, 1, xv23)
                load_block(xqT, 1, xq1)
                nc.sync.dma_start(wk_sb[:, :, P:DKV], wkr[:, :, P:DKV])
                nc.sync.dma_start(wq_sb[:, :, P:DKV], wqr[:, :, P:DKV])
                nc.sync.dma_start(wo_sb, wo.rearrange("(c p) n -> p c n", p=P))

            # Head: K/Q projections for the first i/j blocks, then the first
            # scores+exps as soon as they land.
            with nc.named_scope("proj_head"):
                proj_qk_pass(xk0, wk_sb, KT, 0, 0)
                proj_qk_pass(xk0, wk_sb, KT, 0, 1)
                proj_qk_pass(xq0, wq_sb, QT, 0, 0)
            ctx_e0 = ctx_ps.tile([P, 512], F32, tag="ctx", name="ctx_e")
            ctx_o0 = ctx_ps.tile([P, 512], F32, tag="ctx", name="ctx_o")
            with nc.named_scope("attn_i0_p0"):
                def vpass(jc):
                    xts = xv01 if jc < 8 else xv23
                    return lambda: proj_v_pass(xts, jc)

                head_extras = {jc: [vpass(jc + 1)] for jc in range(15)}
                # K/Q projection passes are placed one block ahead of their
                # first (pumped) consumer, earliest-deadline first.
                qk_halves(head_extras, 1, xq0, wq_sb, QT, 0, 1)
                qk_halves(head_extras, 5, xk1, wk_sb, KT, 0, 2)
                qk_halves(head_extras, 7, xk1, wk_sb, KT, 0, 3)
                # scores/exp for jc 0..1 go ahead of the V0 pass (which
                # waits on the later xv stream) so exps start at the QT
                # block-0 arrival.
                pump()
                pump()
                proj_v_pass(xv01, 0)
                attn_jcs(0, 0, ctx_e0, ctx_o0, head_extras, base_unit=0)
                attn_norm(0, 0, ctx_e0, ctx_o0)

            ALLJC = (3, 5, 7, 9, 11, 13, 14, 15)
            b10_extras = qk_halves({}, 1, xq1, wq_sb, QT, 0, 2)
            b20_extras = qk_halves({}, 1, xq1, wq_sb, QT, 0, 3)
            qk_halves(b20_extras, 5, xk0, wk_sb, KT, 1, 0)
            qk_halves(b20_extras, 7, xk0, wk_sb, KT, 1, 1)
            b30_extras = qk_halves({}, 1, xq0, wq_sb, QT, 1, 0)
            qk_halves(b30_extras, 5, xk1, wk_sb, KT, 1, 2)
            qk_halves(b30_extras, 7, xk1, wk_sb, KT, 1, 3)
            b01_extras = qk_halves({}, 1, xq0, wq_sb, QT, 1, 1)
            b11_extras = wo_sched(
                qk_halves({}, 1, xq1, wq_sb, QT, 1, 2), 0, 0, 8, jcs=ALLJC
            )
            b21_extras = wo_sched(
                qk_halves({}, 1, xq1, wq_sb, QT, 1, 3), 1, 0, 8, jcs=ALLJC
            )
            attn_block(1, 0, extras=b10_extras)
            attn_block(2, 0, extras=b20_extras)
            attn_block(3, 0, extras=b30_extras)
            attn_block(0, 1, extras=b01_extras)
            attn_block(1, 1, extras=b11_extras)
            attn_block(2, 1, extras=b21_extras)
            attn_block(3, 1, extras=wo_sched({}, 2, 0, 8, jcs=ALLJC), pe_tr=True)
            with nc.named_scope("wo_tail"):
                # Last i block's pieces ping-pong psum banks and copy
                # engines so the drain pipelines across DVE and the (by now
                # idle) scalar engine.
                for n in range(8):
                    wo_piece(12 + n // 2, n % 2, alt=(n % 2 == 1))
    nc.compile()
    return nc


def get_nc():
    if not _NC_CACHE:
        _NC_CACHE.append(_build_nc())
    return _NC_CACHE[0]


def kernel(query, key, value, mask, Wq, Wk, Wv, Wo, **_run_kwargs):
    query = np.asarray(query, np.float32)
    key = np.asarray(key, np.float32)
    value = np.asarray(value, np.float32)
    Wq = np.asarray(Wq, np.float32)
    Wk = np.asarray(Wk, np.float32)
    Wv = np.asarray(Wv, np.float32)
    Wo = np.asarray(Wo, np.float32)

    nc = get_nc()
    bf = ml_dtypes.bfloat16
    in_maps = []
    for b in range(2):
        xqT = np.ascontiguousarray(query[b].T).astype(bf)
        xkT = np.ascontiguousarray(key[b].T).astype(bf)
        xvT = np.ascontiguousarray(value[b].T).astype(bf)
        for g in range(4):
            c0 = g * DKV
            in_maps.append(
                {
                    "xqT": xqT,
                    "xkT": xkT,
                    "xvT": xvT,
                    "wq": np.ascontiguousarray(Wq[:, c0 : c0 + DKV]).astype(bf),
                    "wk": np.ascontiguousarray(Wk[:, c0 : c0 + DKV]).astype(bf),
                    "wv": np.ascontiguousarray(Wv[:, c0 : c0 + DKV]).astype(bf),
                    "wo": np.ascontiguousarray(Wo[c0 : c0 + DKV, :]).astype(bf),
                }
            )
    res = run_bass_kernel_spmd(nc, in_maps, core_ids=list(range(8)), **_run_kwargs)
    outs = [r["out"].astype(np.float32) for r in res.results]
    full = np.stack(
        [
            outs[0] + outs[1] + outs[2] + outs[3],
            outs[4] + outs[5] + outs[6] + outs[7],
        ]
    )
    if _run_kwargs:
        return full, res
    return full


# revision 40
# speedup vs baseline: 1.0644x; 1.0644x over previous
"""Multi-head attention TRN2 kernel.

Full inputs -> 8-core shard (batch x head-group) -> Bass/Tile kernel -> host
gather+reduce.

Problem (hardcoded): B=2, S=2048, D_MODEL=1024, H=16, Dk=64, fp32, mask=0.

Sharding: core c = b*4 + g handles batch b and head group g (4 heads).
Each core computes, for its batch's full sequence:
  QT/KT = (x @ Wq_g)^T, V = x @ Wv_g            (x transposed on host, bf16)
  scoresT[j,i] = K Q^T  per head pair            (row-tiled on the PE array)
  attnT = exp(scoresT/8)                         (scalar engine, psum->bf16)
  ctx[i,dk]  = sum_j attnT V: attn chunks are the STATIONARY operand and the
               (dk+ones) V columns are the 65-row moving operand, so the PE
               pays 65 rows per 128x128 attn chunk instead of 512.  The ones
               column makes the softmax denominator ride along in psum.
  normalize: DVE tensor_scalar multiply by the per-partition reciprocal of
             the denominator column (fused with the psum->sbuf move).
  ctx^T via DMA-engine transposes (bf16 xbar path, zero PE cost).
  partial_out = ctxT^T @ Wo_g -> bf16 partials, host sums the 4 group
               partials per batch in f32.
All matmul moving operands are bf16 (1 PE cycle/row in the cost model at any
size).  psum->sbuf moves for projections and the output ride on the
otherwise-idle Pool (gpsimd) engine so the DVE only runs the normalize.
"""

import sys

import ml_dtypes
import numpy as np

try:
    import concourse.bass as bass  # noqa: F401
except ImportError:  # harness runs from a bare directory
    sys.path.insert(0, "/opt/trn_rl_repo")
    import concourse.bass as bass  # noqa: F401

import concourse.tile as tile
from concourse import bacc, mybir
from concourse.bass_utils import run_bass_kernel_spmd
from concourse.masks import make_identity

S = 2048
D = 1024
HG = 4  # heads per core
DK = 64
DKV = HG * DK  # 256
P = 128
F32 = mybir.dt.float32
BF16 = mybir.dt.bfloat16
EXP = mybir.ActivationFunctionType.Exp

_NC_CACHE = []


def _build_nc():
    nc = bacc.Bacc("TRN2", target_bir_lowering=False, debug=False)
    xqT = nc.dram_tensor("xqT", [D, S], BF16, kind="ExternalInput")
    xkT = nc.dram_tensor("xkT", [D, S], BF16, kind="ExternalInput")
    xvT = nc.dram_tensor("xvT", [D, S], BF16, kind="ExternalInput")
    wq = nc.dram_tensor("wq", [D, DKV], BF16, kind="ExternalInput")
    wk = nc.dram_tensor("wk", [D, DKV], BF16, kind="ExternalInput")
    wv = nc.dram_tensor("wv", [D, DKV], BF16, kind="ExternalInput")
    wo = nc.dram_tensor("wo", [DKV, D], BF16, kind="ExternalInput")
    out = nc.dram_tensor("out", [S, D], BF16, kind="ExternalOutput")

    with tile.TileContext(nc) as tc:
        with (
            tc.tile_pool(name="persist", bufs=1) as persist,
            tc.tile_pool(name="xin", bufs=40) as xin,
            tc.tile_pool(name="attn", bufs=29) as attn_pool,
            tc.tile_pool(name="small", bufs=8) as small_pool,
            tc.tile_pool(name="hid", bufs=6) as hid_pool,
            tc.tile_pool(name="sc_ps", bufs=2, space="PSUM") as sc_ps,
            tc.tile_pool(name="ctx_ps", bufs=2, space="PSUM") as ctx_ps,
            tc.tile_pool(name="wo_ps", bufs=2, space="PSUM") as wo_ps,
        ):
            # Persistent SBUF tensors.
            # QT/KT: [dkv%128, pair, i] -- partition r = (h%2)*64 + dk.
            QT = persist.tile([P, 2, S], BF16)
            KT = persist.tile([P, 2, S], BF16)
            # V augmented with a ones column (-> softmax denominator rides
            # along in the ctx matmul): [j%128, jc, head, dk+1].
            Vag = persist.tile([P, 16, HG, DK + 1], BF16)
            # Normalized ctx in query-major layout: [i%128, ichunk, pair,
            # head01*64+dk]; transposed (DMA xbar) into ctxT [dkv%128, pair, i]
            ctxN = persist.tile([P, 16, 2, P], BF16)
            ctxT = persist.tile([P, 2, S], BF16)
            wq_sb = persist.tile([P, 8, DKV], BF16)
            wk_sb = persist.tile([P, 8, DKV], BF16)
            wv_sb = persist.tile([P, 8, DKV], BF16)
            wo_sb = persist.tile([P, 2, D], BF16)

            # Preload the exp table set (~1.3us) during the DMA head.
            warm = small_pool.tile([P, 8], F32, tag="warm")
            nc.vector.memset(warm[0:1, :], 0.0)
            nc.scalar.activation(warm[0:1, :], warm[0:1, :], EXP, scale=0.0)
            # Ones column of Vag.
            nc.vector.memset(Vag[:, :, :, DK], 1.0)
            # Identity for the last block's PE-path transposes.
            ident = persist.tile([P, P], BF16)
            make_identity(nc, ident)

            def alloc_block(name):
                """Reserve the 8 k-chunk tiles for one 1024-wide block.
                The xv blocks are allocated FIRST (before the longer-lived
                xk/xq blocks) so the pool's FIFO reuse hands their slots --
                dead after block (0,0)'s V passes -- to the tail of the xk1/
                xq1 streams instead of stalling on pair-1-pinned chunks."""
                return [
                    xin.tile([P, 1024], BF16, tag="xt", name=f"{name}{k}")
                    for k in range(8)
                ]

            def load_block(xT, icb, xts):
                """Issue the 8 k-chunk DMAs for one 1024-wide column block."""
                for k in range(8):
                    nc.sync.dma_start(
                        xts[k],
                        xT[k * P : (k + 1) * P, icb * 1024 : (icb + 1) * 1024],
                    )
                return xts

            # [pair][512-block] flags marking which Q/K projection passes
            # have been EMITTED -- the exp pump must not run ahead of them
            # (an in-order PE stream would deadlock on the missing pass).
            qt_done = [[False] * 4 for _ in range(2)]
            kt_done = [[False] * 4 for _ in range(2)]

            def proj_qk_pass(xts, w_sb, OUT, pair, ib, ks=range(8), st=None,
                             do_pump=False):
                """Project one (pair, 512-wide i block) of x into OUT.
                do_pump interleaves one scores+exp pump after each matmul:
                the pass's matmuls pace with their input DMA stream, and the
                pumps fill the arrival gaps (sc psum is disjoint from the
                pass's accumulator pool, so the interleave is ring-safe)."""
                icb, ic2 = ib // 2, ib % 2
                if st is None or "acc" not in st:
                    acc = wo_ps.tile([P, 512], F32, tag="wo", name="pacc")
                    if st is not None:
                        st["acc"] = acc
                else:
                    acc = st.pop("acc")
                for k in ks:
                    nc.tensor.matmul(
                        acc,
                        w_sb[:, k, pair * P : (pair + 1) * P],
                        xts[k][:, ic2 * 512 : (ic2 + 1) * 512],
                        start=(k == 0),
                        stop=(k == 7),
                    )
                    if do_pump and k >= 1:
                        pump()
                if ks[-1] == 7:
                    o0 = ib * 512
                    nc.vector.tensor_copy(OUT[:, pair, o0 : o0 + 512], acc)
                    done = qt_done if OUT is QT else kt_done
                    done[pair][ib] = True

            def qk_halves(extras, jc, xts, w_sb, OUT, pair, ib):
                """Weave one projection pass as two 4-matmul halves in
                adjacent jc slots (the half-open psum accumulator tolerates
                at most one other pool allocation in between, which the
                placements below respect)."""
                st = {}
                extras.setdefault(jc, []).append(
                    lambda: proj_qk_pass(xts, w_sb, OUT, pair, ib, range(4), st)
                )
                extras.setdefault(jc + 1, []).append(
                    lambda: proj_qk_pass(xts, w_sb, OUT, pair, ib, range(4, 8), st)
                )
                return extras

            def proj_v_pass(xts, jc):
                """Project one 128-wide j chunk of value into Vag[:, jc]."""
                c0 = (jc % 8) * P
                acc = wo_ps.tile([P, 512], F32, tag="wo", name="vacc")
                for k in range(8):
                    nc.tensor.matmul(
                        acc[:, 0:DKV],
                        xts[k][:, c0 : c0 + P],
                        wv_sb[:, k, :],
                        start=(k == 0),
                        stop=(k == 7),
                    )
                    if k >= 1:
                        pump()
                nc.vector.tensor_copy(
                    Vag[:, jc, :, 0:DK],
                    acc[:, 0:DKV].rearrange("p (h d) -> p h d", h=HG),
                )

            def emit_scores(ic, pair, jc):
                """scoresT for the head pair (row-tiled, concurrent) + exp."""
                i0 = ic * 512
                j0 = jc * P
                sc = sc_ps.tile([P, 1024], F32, tag="sc", name="sc")
                nc.tensor.matmul(
                    sc[:, 0:512],
                    KT[0:64, pair, j0 : j0 + P],
                    QT[0:64, pair, i0 : i0 + 512],
                    start=True,
                    stop=True,
                )
                nc.tensor.matmul(
                    sc[:, 512:1024],
                    KT[64:128, pair, j0 : j0 + P],
                    QT[64:128, pair, i0 : i0 + 512],
                    start=True,
                    stop=True,
                )
                at = attn_pool.tile([P, 1024], BF16, tag="at")
                nc.scalar.activation(at, sc, EXP, scale=0.125)
                return at

            def emit_ctx(ctx_e, ctx_o, pair, jc, at):
                """ctx accumulation: attn chunks stationary, (V|1) moving.

                psum region sub*65..sub*65+65 of ctx_e/ctx_o holds the i
                sub-chunk's (dk + denominator) columns.  Only the first
                region's jc==0 matmul uses start=True: it zeroes the whole
                2KB psum bank, which covers the other three regions (trn2
                zero-region == bank)."""
                for head01, bank in ((0, ctx_e), (1, ctx_o)):
                    h = 2 * pair + head01
                    for sub in range(4):
                        first = jc == 0 and sub == 0
                        nc.tensor.matmul(
                            bank[:, sub * 65 : sub * 65 + 65],
                            at[:, head01 * 512 + sub * P : head01 * 512 + (sub + 1) * P],
                            Vag[:, jc, h, :],
                            start=first,
                            stop=(jc == 15 and sub == 3),
                            skip_group_check=not first,
                        )

            # Decoupled exp stream: scores+exp for unit u (block-major order)
            # are emitted ("pumped") up to MAXLEAD units ahead of the ctx
            # cursor, so the scalar engine streams exps continuously through
            # the PE-bound projection phase instead of running lockstep with
            # ctx.  The at ring (bufs > MAXLEAD) buffers the lead; psum sc
            # ring pacing self-regulates PE emission against ACT execution.
            # Pair-major block order: all pair-0 blocks run first, so the
            # pair-1 K/Q projection passes move out of the PE-bound front
            # into the later ACT-bound blocks where the PE has slack.
            BLOCKS = [(ic, 0) for ic in range(4)] + [(ic, 1) for ic in range(4)]
            UNITS = [
                (ic, pair, jc) for ic, pair in BLOCKS for jc in range(16)
            ]
            MAXLEAD = 26
            exp_cur = [0]
            ctx_done = [0]
            pending_at = {}

            def unit_ready(u):
                ic, p, jc = UNITS[u]
                return qt_done[p][ic] and kt_done[p][jc // 4]

            def pump():
                u = exp_cur[0]
                if (
                    u >= len(UNITS)
                    or u >= ctx_done[0] + MAXLEAD
                    or not unit_ready(u)
                ):
                    return False
                pending_at[UNITS[u]] = emit_scores(*UNITS[u])
                exp_cur[0] += 1
                return True

            def attn_jcs(ic, pair, ctx_e, ctx_o, extras=None, base_unit=0,
                         pre=None):
                extras = extras or {}
                pre = pre or {}
                for jc in range(16):
                    u = base_unit + jc
                    ctx_done[0] = u
                    while exp_cur[0] <= u:
                        assert pump(), f"pump stuck at unit {exp_cur[0]}"
                    # 3 pumps/slot through the PE-bound projection front so
                    # the scalar engine stays fed; 2 in steady state.
                    budget = 3 if u < 32 else 2
                    while budget and pump():
                        budget -= 1
                    for fn in pre.get(jc, ()):
                        fn()
                    at = pending_at.pop((ic, pair, jc))
                    emit_ctx(ctx_e, ctx_o, pair, jc, at)
                    for fn in extras.get(jc, ()):
                        fn()

            def attn_norm(ic, pair, ctx_e, ctx_o, pe_tr=False):
                """softmax normalize: per-partition reciprocal of the psum
                denominator columns (one strided op per bank), broadcast-
                multiplied while moving the ctx psum regions to ctxN (bf16);
                then transpose into ctxT.  Transposes ride the DMA xbar path
                (zero PE cost) except for the last block, where the shorter
                PE-transpose latency chain trims the kernel tail."""
                for head01, bank in ((0, ctx_e), (1, ctx_o)):
                    regs = bank[:, 0 : 4 * 65].rearrange("p (s c) -> p s c", c=65)
                    rc4 = small_pool.tile([P, 4], F32, tag="rc")
                    nc.vector.reciprocal(rc4, regs[:, :, DK])
                    with nc.allow_low_precision("bf16 ctx storage"):
                        nc.vector.tensor_mul(
                            ctxN[
                                :,
                                ic * 4 : ic * 4 + 4,
                                pair,
                                head01 * DK : (head01 + 1) * DK,
                            ],
                            regs[:, :, 0:DK],
                            rc4.unsqueeze(2).broadcast_to([P, 4, DK]),
                        )
                if pe_tr:
                    tr = ctx_ps.tile([P, 512], F32, tag="ctx", name="tr")
                    tr_bf = tr.bitcast(BF16)
                    for sub in range(4):
                        nc.tensor.transpose(
                            tr_bf[:, sub * P : (sub + 1) * P],
                            ctxN[:, ic * 4 + sub, pair, :],
                            ident,
                        )
                    nc.vector.tensor_copy(
                        ctxT[:, pair, ic * 512 : (ic + 1) * 512], tr_bf[:, 0:512]
                    )
                    return
                for sub in range(4):
                    ichunk = ic * 4 + sub
                    nc.sync.dma_start_transpose(
                        ctxT[:, pair, ichunk * P : (ichunk + 1) * P],
                        ctxN[:, ichunk, pair, :],
                    )

            def attn_block(ic, pair, extras=None, pe_tr=False, pre=None):
                with nc.named_scope(f"attn_i{ic}_p{pair}"):
                    ctx_e = ctx_ps.tile([P, 512], F32, tag="ctx", name="ctx_e")
                    ctx_o = ctx_ps.tile([P, 512], F32, tag="ctx", name="ctx_o")
                    attn_jcs(
                        ic, pair, ctx_e, ctx_o, extras,
                        base_unit=BLOCKS.index((ic, pair)) * 16, pre=pre,
                    )
                    attn_norm(ic, pair, ctx_e, ctx_o, pe_tr=pe_tr)

            hs_tiles = {}

            def wo_piece(ichunk, d2, alt=False):
                """One output-projection piece.  The two d halves of an i
                chunk share one [P, 1024] staging tile and a single full-row
                output DMA (2KB contiguous rows).  alt=True (tail only)
                draws the psum accumulator from the retired scores pool and
                runs the psum->sbuf move on the (by then idle) scalar
                engine, so the tail drain pipelines across two psum banks
                and two copy engines."""
                r0 = ichunk * P
                if alt:
                    hp = sc_ps.tile([P, 1024], F32, tag="sc", name="hp")[:, 0:512]
                else:
                    hp = wo_ps.tile([P, 512], F32, tag="wo", name="hp")
                nc.tensor.matmul(
                    hp,
                    ctxT[:, 0, r0 : r0 + P],
                    wo_sb[:, 0, d2 * 512 : (d2 + 1) * 512],
                    start=True,
                    stop=False,
                )
                nc.tensor.matmul(
                    hp,
                    ctxT[:, 1, r0 : r0 + P],
                    wo_sb[:, 1, d2 * 512 : (d2 + 1) * 512],
                    start=False,
                    stop=True,
                )
                if d2 == 0:
                    hs_tiles[ichunk] = hid_pool.tile(
                        [P, 1024], BF16, tag="hs", name="hs"
                    )
                hs = hs_tiles.pop(ichunk) if d2 == 1 else hs_tiles[ichunk]
                if alt:
                    nc.scalar.copy(hs[:, d2 * 512 : (d2 + 1) * 512], hp)
                else:
                    nc.vector.tensor_copy(hs[:, d2 * 512 : (d2 + 1) * 512], hp)
                if d2 == 1:
                    nc.sync.dma_start(out[r0 : r0 + P, :], hs)

            def wo_sched(extras, ic, lo, hi, jcs=(3, 7, 11, 15)):
                """Weave pieces lo..hi of i block ic at the given jc slots
                (the jc15 slot doubles as the block-boundary filler that
                covers the normalize latency)."""
                for n, jc in zip(range(lo, hi), jcs):
                    extras.setdefault(jc, []).append(
                        lambda ichunk=ic * 4 + n // 2, d2=n % 2: wo_piece(ichunk, d2)
                    )
                return extras

            # ---------------- emission schedule ----------------
            # ALL input DMAs are issued up front in arrival-priority order.
            # The xin ring (20 slots) paces them against consumption.
            with nc.named_scope("loads"):
                # Pair-0 weight halves lead the stream (they gate the first
                # projections); pair-1 halves are not consumed until the
                # pair-1 blocks in the back half of the kernel.
                xv01 = alloc_block("xv01_")
                xv23 = alloc_block("xv23_")
                xk0 = alloc_block("xk0_")
                xq0 = alloc_block("xq0_")
                xk1 = alloc_block("xk1_")
                xq1 = alloc_block("xq1_")
                wkr = wk.rearrange("(ko p) n -> p ko n", p=P)
                wqr = wq.rearrange("(ko p) n -> p ko n", p=P)
                nc.sync.dma_start(wk_sb[:, :, 0:P], wkr[:, :, 0:P])
                load_block(xkT, 0, xk0)
                nc.sync.dma_start(wq_sb[:, :, 0:P], wqr[:, :, 0:P])
                load_block(xqT, 0, xq0)
                load_block(xkT, 1, xk1)
                nc.sync.dma_start(wv_sb, wv.rearrange("(ko p) n -> p ko n", p=P))
                load_block(xvT, 0, xv01)
                load_block(xvT, 1, xv23)
                load_block(xqT, 1, xq1)
                nc.sync.dma_start(wk_sb[:, :, P:DKV], wkr[:, :, P:DKV])
                nc.sync.dma_start(wq_sb[:, :, P:DKV], wqr[:, :, P:DKV])
                nc.sync.dma_start(wo_sb, wo.rearrange("(c p) n -> p c n", p=P))

            # Head: K/Q projections for the first i/j blocks, then the first
            # scores+exps as soon as they land.
            with nc.named_scope("proj_head"):
                proj_qk_pass(xk0, wk_sb, KT, 0, 0)
                proj_qk_pass(xk0, wk_sb, KT, 0, 1)
                proj_qk_pass(xq0, wq_sb, QT, 0, 0)
                pump()
                pump()
                proj_qk_pass(xk1, wk_sb, KT, 0, 2, do_pump=True)
                proj_qk_pass(xk1, wk_sb, KT, 0, 3, do_pump=True)
                proj_qk_pass(xq0, wq_sb, QT, 0, 1, do_pump=True)
            ctx_e0 = ctx_ps.tile([P, 512], F32, tag="ctx", name="ctx_e")
            ctx_o0 = ctx_ps.tile([P, 512], F32, tag="ctx", name="ctx_o")
            with nc.named_scope("attn_i0_p0"):
                def vpass(jc):
                    xts = xv01 if jc < 8 else xv23
                    return lambda: proj_v_pass(xts, jc)

                head_pre = {jc: [vpass(jc)] for jc in range(16)}
                head_extras = qk_halves({}, 9, xq1, wq_sb, QT, 0, 2)
                attn_jcs(0, 0, ctx_e0, ctx_o0, head_extras, base_unit=0,
                         pre=head_pre)
                attn_norm(0, 0, ctx_e0, ctx_o0)

            ALLJC = (3, 5, 7, 9, 11, 13, 14, 15)
            b10_extras = qk_halves({}, 1, xq1, wq_sb, QT, 0, 3)
            b20_extras = qk_halves({}, 1, xq0, wq_sb, QT, 1, 0)
            qk_halves(b20_extras, 5, xk0, wk_sb, KT, 1, 0)
            qk_halves(b20_extras, 7, xk0, wk_sb, KT, 1, 1)
            b30_extras = qk_halves({}, 5, xk1, wk_sb, KT, 1, 2)
            qk_halves(b30_extras, 7, xk1, wk_sb, KT, 1, 3)
            b01_extras = qk_halves({}, 1, xq0, wq_sb, QT, 1, 1)
            b11_extras = wo_sched(
                qk_halves({}, 1, xq1, wq_sb, QT, 1, 2), 0, 0, 8, jcs=ALLJC
            )
            b21_extras = wo_sched(
                qk_halves({}, 1, xq1, wq_sb, QT, 1, 3), 1, 0, 8, jcs=ALLJC
            )
            attn_block(1, 0, extras=b10_extras)
            attn_block(2, 0, extras=b20_extras)
            attn_block(3, 0, extras=b30_extras)
            attn_block(0, 1, extras=b01_extras)
            attn_block(1, 1, extras=b11_extras)
            attn_block(2, 1, extras=b21_extras)
            attn_block(3, 1, extras=wo_sched({}, 2, 0, 8, jcs=ALLJC), pe_tr=True)
            with nc.named_scope("wo_tail"):
                # Last i block's pieces ping-pong psum banks and copy
                # engines so the drain pipelines across DVE and the (by now
                # idle) scalar engine.
                for n in range(8):
                    wo_piece(12 + n // 2, n % 2, alt=(n % 2 == 1))
    nc.compile()
    return nc


def get_nc():
    if not _NC_CACHE:
        _NC_CACHE.append(_build_nc())
    return _NC_CACHE[0]


def kernel(query, key, value, mask, Wq, Wk, Wv, Wo, **_run_kwargs):
    query = np.asarray(query, np.float32)
    key = np.asarray(key, np.float32)
    value = np.asarray(value, np.float32)
    Wq = np.asarray(Wq, np.float32)
    Wk = np.asarray(Wk, np.float32)
    Wv = np.asarray(Wv, np.float32)
    Wo = np.asarray(Wo, np.float32)

    nc = get_nc()
    bf = ml_dtypes.bfloat16
    in_maps = []
    for b in range(2):
        xqT = np.ascontiguousarray(query[b].T).astype(bf)
        xkT = np.ascontiguousarray(key[b].T).astype(bf)
        xvT = np.ascontiguousarray(value[b].T).astype(bf)
        for g in range(4):
            c0 = g * DKV
            in_maps.append(
                {
                    "xqT": xqT,
                    "xkT": xkT,
                    "xvT": xvT,
                    "wq": np.ascontiguousarray(Wq[:, c0 : c0 + DKV]).astype(bf),
                    "wk": np.ascontiguousarray(Wk[:, c0 : c0 + DKV]).astype(bf),
                    "wv": np.ascontiguousarray(Wv[:, c0 : c0 + DKV]).astype(bf),
                    "wo": np.ascontiguousarray(Wo[c0 : c0 + DKV, :]).astype(bf),
                }
            )
    res = run_bass_kernel_spmd(nc, in_maps, core_ids=list(range(8)), **_run_kwargs)
    outs = [r["out"].astype(np.float32) for r in res.results]
    full = np.stack(
        [
            outs[0] + outs[1] + outs[2] + outs[3],
            outs[4] + outs[5] + outs[6] + outs[7],
        ]
    )
    if _run_kwargs:
        return full, res
    return full
